# revision 7
# baseline (speedup 1.0000x reference)
"""Trainium2 Bass kernel for the GCNN layer (nn_GCNNLayer_71536975282326).

out = relu( einsum('nd,nde->ne', x, W_pos) + b_pos
            + einsum('nre,nr->ne', einsum('nd,rde->nre', x, W_dep), counts)
            + counts @ b_dep )
with counts[n,r] = #edges (token n, type r).

Strategy (8 NeuronCores, SPMD, one program):
  - Shard the R=92 W_dep stack across cores (12 slots/core, zero-padded) and
    the N=150 W_pos stack across cores (19 slots/core, zero-padded).
  - All matmuls run in float32r (fp32 storage, full-rate PE) — measured
    ~1.5e-4 scale-relative error vs 3e-7 for fp32, at 4x the PE throughput.
  - Dep accumulation is transposed, out_T[e, n], in 4 PSUM banks (two 256-wide
    token regions per bank, one per 128-row e-chunk). Moving operand is the
    host-prescaled (counts[:,r]*x)^T, padded to 256 columns so f32r runs at
    1 cycle/row.
  - Self term: per token, 16 matmuls W_pos[n_j] chunks (moving, 512-wide)
    against the local x^T column block (stationary [128,19]); out rows land in
    a [19,512] PSUM tile; row j is extracted to SBUF. Avoids the 1216
    N=1 matvecs whose systolic-flush overhead dominated v1.
  - Self units run FIRST so their AllGather overlaps the dep DMA stream.
  - Bias: one K=32 f32 matmul per e-chunk: lhsT rows = [b_dep slice ; b_pos
    rows], rhs = [counts slice^T ; one-hot placing token n_j at column n_j].
  - AllReduce the [1024,150] main partial; AllGather the [19,1024] self
    partials (slot (core k, j) IS global token 19k+j); add + relu on device;
    host transposes the [1024,150] result.
"""

import numpy as np

import concourse.bass as bass
import concourse.tile as tile
from concourse import bacc, mybir
from concourse.bass_utils import run_bass_kernel_spmd

N, D, R = 150, 1024, 92
NCORES = 8
P = 128
DC = D // P            # 8 contraction (d) chunks
EC = D // P            # 8 output (e) chunks
NB = EC // 2           # 4 main psum banks, two e-chunks each
NPAD = 256             # token axis padded so f32r moving >= 256
DEP_SLOTS = 12         # ceil(92/8)
SELF_SLOTS = 19        # ceil(150/8)
KAUG = 32              # 12 dep-count rows + 19 one-hot rows + 1 pad
F32 = mybir.dt.float32
F32R = mybir.dt.float32r

DEP_SPLIT = [12, 12, 12, 12, 11, 11, 11, 11]
DEP_STARTS = np.concatenate([[0], np.cumsum(DEP_SPLIT)])

_PROG = None


def _build_program():
    nc = bacc.Bacc("TRN2", target_bir_lowering=False, debug=False, num_devices=NCORES)

    wdep = nc.dram_tensor("wdep", [DEP_SLOTS, D, D], F32R, kind="ExternalInput")
    wpos = nc.dram_tensor("wpos", [SELF_SLOTS, D, D], F32R, kind="ExternalInput")
    xs = nc.dram_tensor("xs", [DEP_SLOTS, DC, P, N], F32R, kind="ExternalInput")
    xtl = nc.dram_tensor("xtl", [DC, P, SELF_SLOTS], F32R, kind="ExternalInput")
    baug = nc.dram_tensor("baug", [KAUG, D], F32, kind="ExternalInput")
    caug = nc.dram_tensor("caug", [KAUG, NPAD], F32, kind="ExternalInput")
    out_T = nc.dram_tensor("out_T", [D, N], F32, kind="ExternalOutput")

    groups = [list(range(NCORES))]

    with tile.TileContext(nc) as tc:
        with (
            tc.tile_pool(name="wpool", bufs=4) as wpool,
            tc.tile_pool(name="xspool", bufs=1) as xspool,
            tc.tile_pool(name="constp", bufs=1) as constp,
            tc.tile_pool(name="mainps", bufs=1, space=bass.MemorySpace.PSUM) as mainps,
            tc.tile_pool(name="selfps", bufs=4, space=bass.MemorySpace.PSUM) as selfps,
            tc.tile_pool(name="dram", bufs=1, space="DRAM") as dram,
            tc.tile_pool(name="fin", bufs=3) as fin,
        ):
            xtl_t = constp.tile([P, DC * SELF_SLOTS], F32R)
            nc.sync.dma_start(
                out=xtl_t.rearrange("p (c j) -> p c j", c=DC),
                in_=xtl[:].rearrange("c p j -> p c j"),
            )
            baug_t = constp.tile([KAUG, D], F32)
            nc.sync.dma_start(out=baug_t[:], in_=baug[:])
            caug_t = constp.tile([KAUG, NPAD], F32)
            nc.sync.dma_start(out=caug_t[:], in_=caug[:])

            accs = [
                mainps.tile([P, 2 * NPAD], F32, name=f"acc{b}", tag=f"acc{b}")
                for b in range(NB)
            ]
            # Bias matmuls first: the single start=True per main PSUM bank (the
            # second region's first-touch rides the bank's pending-zero state).
            for b in range(NB):
                for h in range(2):
                    nc.tensor.matmul(
                        accs[b][:, h * NPAD : h * NPAD + NPAD],
                        baug_t[:, (2 * b + h) * P : (2 * b + h + 1) * P],
                        caug_t[:],
                        start=(h == 0),
                        stop=False,
                    )

            def load_w(src, u):
                wt = wpool.tile([P, DC * D], F32R, tag="w", name=f"w{u}")
                src3 = src.rearrange("(c p) e -> p c e", p=P)
                wt3 = wt.rearrange("p (c e) -> p c e", c=DC)
                for g in range(2):
                    nc.sync.dma_start(
                        out=wt3[:, 4 * g : 4 * g + 4, :],
                        in_=src3[:, 4 * g : 4 * g + 4, :],
                    )
                return wt

            # ---- self phase: 19 tokens, M=1 row formulation ----
            # Each token's result row is computed at partition 0 of a small
            # PSUM tile (f32r cost is moving rows only, M is irrelevant),
            # bounced through SBUF (engines cannot address partition j>0),
            # and DMAed into its ar_self_in row.
            ar_self_in = dram.tile([SELF_SLOTS, D], F32)
            ar_self_out = dram.tile([NCORES, SELF_SLOTS, D], F32, addr_space="Shared")
            for j in range(SELF_SLOTS):
                wt = load_w(wpos[j], f"s{j}")
                for eh in range(2):
                    st = selfps.tile([1, 512], F32, tag="sp", name=f"sp{j}_{eh}")
                    for c in range(DC):
                        nc.tensor.matmul(
                            st[:],
                            xtl_t[:, c * SELF_SLOTS + j : c * SELF_SLOTS + j + 1],
                            wt[:, c * D + eh * 512 : c * D + (eh + 1) * 512],
                            start=(c == 0),
                            stop=(c == DC - 1),
                        )
                    sx = fin.tile([1, 512], F32, tag="sx", name=f"sx{j}_{eh}")
                    nc.scalar.copy(out=sx[:], in_=st[:])
                    nc.sync.dma_start(
                        out=ar_self_in[j : j + 1, eh * 512 : (eh + 1) * 512],
                        in_=sx[:],
                    )

            nc.gpsimd.collective_compute(
                "AllGather", mybir.AluOpType.bypass,
                replica_groups=groups, ins=[ar_self_in.opt()], outs=[ar_self_out.opt()],
            )

            # ---- dep phase: 12 type slots ----
            # xs tiles are pre-zeroed once; DMA refills only the first 150 of
            # each 256-wide chunk region, the zero padding is never rewritten.
            NXS = 3
            xsts = [xspool.tile([P, DC * NPAD], F32R, tag=f"xs{i}", name=f"xs{i}") for i in range(NXS)]
            for t in xsts:
                nc.vector.memset(t[:].bitcast(F32), 0.0)
            for i in range(DEP_SLOTS):
                wt = load_w(wdep[i], f"d{i}")
                xst = xsts[i % NXS]
                nc.sync.dma_start(
                    out=xst.rearrange("p (c f) -> p c f", c=DC)[:, :, 0:N],
                    in_=xs[i].rearrange("c p f -> p c f"),
                )
                last = i == DEP_SLOTS - 1
                for c in range(DC):
                    for ec in range(EC):
                        b, h = divmod(ec, 2)
                        nc.tensor.matmul(
                            accs[b][:, h * NPAD : h * NPAD + NPAD],
                            wt[:, c * D + ec * P : c * D + (ec + 1) * P],
                            xst[:, c * NPAD : (c + 1) * NPAD],
                            start=False,
                            stop=last and c == DC - 1 and h == 1,
                        )

            # ---- evacuate + AllReduce ----
            ar_main_in = dram.tile([D, N], F32)
            ar_main_out = dram.tile([D, N], F32, addr_space="Shared")
            for b in range(NB):
                ev = fin.tile([P, 2 * NPAD], F32, tag="ev", name=f"ev{b}")
                nc.vector.tensor_copy(ev[:], accs[b][:])
                for h in range(2):
                    nc.sync.dma_start(
                        out=ar_main_in[(2 * b + h) * P : (2 * b + h + 1) * P, :],
                        in_=ev[:, h * NPAD : h * NPAD + N],
                    )
            nc.gpsimd.collective_compute(
                "AllReduce", mybir.AluOpType.add,
                replica_groups=groups, ins=[ar_main_in.opt()], outs=[ar_main_out.opt()],
            )

            # ---- final combine: out_T[e, n] = relu(main + self) ----
            for ec in range(EC):
                mc = fin.tile([P, N], F32, tag="mc", name=f"mc{ec}")
                nc.sync.dma_start(out=mc[:], in_=ar_main_out[ec * P : (ec + 1) * P, :])
                sc = fin.tile([P, NCORES * SELF_SLOTS], F32, tag="sc", name=f"sc{ec}")
                nc.sync.dma_start(
                    out=sc.rearrange("p (k j) -> p k j", k=NCORES),
                    in_=ar_self_out[:, :, ec * P : (ec + 1) * P].rearrange("k j p -> p k j"),
                )
                oc = fin.tile([P, N], F32, tag="oc", name=f"oc{ec}")
                # self slot (k, j) == global token 19k+j, so cols 0:150 align
                nc.vector.scalar_tensor_tensor(
                    oc[:], mc[:], 0.0, sc[:, 0:N],
                    mybir.AluOpType.add, mybir.AluOpType.add,
                )
                nc.vector.tensor_scalar_max(oc[:], oc[:], 0.0)
                nc.sync.dma_start(out=out_T[ec * P : (ec + 1) * P, :], in_=oc[:])

    nc.compile()
    return nc


def _get_program():
    global _PROG
    if _PROG is None:
        _PROG = _build_program()
    return _PROG


def _prepare_in_maps(x, W_pos, b_pos, W_dep, b_dep, edge_token, edge_type):
    x = np.ascontiguousarray(np.asarray(x, dtype=np.float32))
    W_pos = np.asarray(W_pos, dtype=np.float32)
    b_pos = np.asarray(b_pos, dtype=np.float32)
    W_dep = np.asarray(W_dep, dtype=np.float32)
    b_dep = np.asarray(b_dep, dtype=np.float32)
    edge_token = np.asarray(edge_token)
    edge_type = np.asarray(edge_type)

    counts = np.zeros((N, R), np.float32)
    np.add.at(counts, (edge_token, edge_type), 1.0)
    xT = np.ascontiguousarray(x.T)  # [D, N]

    in_maps = []
    for k in range(NCORES):
        r0, r1 = int(DEP_STARTS[k]), int(DEP_STARTS[k + 1])
        nr = r1 - r0
        t0 = SELF_SLOTS * k
        t1 = min(t0 + SELF_SLOTS, N)
        nt = t1 - t0

        wdep_k = np.zeros((DEP_SLOTS, D, D), np.float32)
        wdep_k[:nr] = W_dep[r0:r1]
        wpos_k = np.zeros((SELF_SLOTS, D, D), np.float32)
        wpos_k[:nt] = W_pos[t0:t1]

        xs_k = np.zeros((DEP_SLOTS, DC, P, N), np.float32)
        for i in range(nr):
            xs_k[i] = (xT * counts[:, r0 + i][None, :]).reshape(DC, P, N)

        xtl_k = np.zeros((DC, P, SELF_SLOTS), np.float32)
        xtl_k[:, :, :nt] = xT[:, t0:t1].reshape(DC, P, nt)

        baug_k = np.zeros((KAUG, D), np.float32)
        baug_k[:nr] = b_dep[r0:r1]
        baug_k[DEP_SLOTS : DEP_SLOTS + nt] = b_pos[t0:t1]

        caug_k = np.zeros((KAUG, NPAD), np.float32)
        caug_k[:nr, 0:N] = counts[:, r0:r1].T
        for j in range(nt):
            caug_k[DEP_SLOTS + j, t0 + j] = 1.0

        in_maps.append(
            dict(wdep=wdep_k, wpos=wpos_k, xs=xs_k, xtl=xtl_k, baug=baug_k, caug=caug_k)
        )
    return in_maps


def _run(in_maps, trace=False):
    nc = _get_program()
    return run_bass_kernel_spmd(nc, in_maps, list(range(NCORES)), trace=trace)


def kernel(x, W_pos, b_pos, W_dep, b_dep, edge_token, edge_type):
    in_maps = _prepare_in_maps(x, W_pos, b_pos, W_dep, b_dep, edge_token, edge_type)
    res = _run(in_maps, trace=False)
    return np.ascontiguousarray(res.results[0]["out_T"].T)


def kernel_traced(x, W_pos, b_pos, W_dep, b_dep, edge_token, edge_type):
    """Like kernel() but with NTFF profiling; returns (output, BassKernelResults)."""
    in_maps = _prepare_in_maps(x, W_pos, b_pos, W_dep, b_dep, edge_token, edge_type)
    res = _run(in_maps, trace=True)
    return np.ascontiguousarray(res.results[0]["out_T"].T), res


def install_ntff_shim():
    """The agent image's antenv lacks axon_hooks; recreate it from the boot
    module's ctypes NTFF driver so run_bass_kernel_spmd(trace=True) can
    capture a neuron-profile. Test-only; kernel() never needs this."""
    import sys
    import types

    try:
        from antenv.axon_hooks import get_axon_ntff_profile_hook  # noqa: F401
        return
    except ImportError:
        pass
    from trn_agent_boot.trn_boot import _ntff_profile_via_ctypes

    hook = _ntff_profile_via_ctypes("/opt/axon/libaxon_pjrt.so")
    mod = types.ModuleType("antenv.axon_hooks")
    mod._hook = hook
    mod.get_axon_ntff_profile_hook = lambda: mod._hook
    mod.set_axon_ntff_profile_hook = lambda h: setattr(mod, "_hook", h)
    sys.modules["antenv.axon_hooks"] = mod


# revision 9
# speedup vs baseline: 1.5497x; 1.5497x over previous
"""Trainium2 Bass kernel for the GCNN layer (nn_GCNNLayer_71536975282326).

out = relu( einsum('nd,nde->ne', x, W_pos) + b_pos
            + einsum('nre,nr->ne', einsum('nd,rde->nre', x, W_dep), counts)
            + counts @ b_dep )
with counts[n,r] = #edges (token n, type r).

Strategy (8 NeuronCores, SPMD, one program):
  - Shard the R=92 W_dep stack across cores (12 slots/core, zero-padded) and
    the N=150 W_pos stack across cores (19 slots/core, zero-padded).
  - All heavy matmuls run in float32r (fp32 storage, ~4x PE rate, measured
    ~1.4e-4 scale-relative error end to end).
  - Dep accumulation is transposed, out_T[e, n], in 4 PSUM banks (two 256-wide
    token regions per bank — f32r needs a >=256-wide moving operand for full
    rate). Moving operand is the host-prescaled (counts[:,r]*x)^T.
  - Self term: per token, 16 M=1 matmuls (W_pos[n_j] chunks moving 512-wide)
    into partition 0 of a small PSUM tile; the row is bounced through SBUF
    (engines cannot address partitions at j>0) into the AllGather input.
    Self units run FIRST so the AllGather overlaps the dep DMA stream.
  - The gathered [152, 1024] self matrix ([token, e]) is transposed back to
    [e, token] ON THE PE via identity matmuls into the freed PSUM banks —
    a strided-DMA transpose would degrade to 4-byte packets and flood the
    DMA engines (measured: 158K single-element packets, +300us).
  - Bias: one K=32 f32 matmul per e-chunk: lhsT rows = [b_dep slice ; b_pos
    rows], rhs = [counts slice^T ; one-hot placing token n_j at column n_j].
  - AllReduce the [1024,150] main partial; add + relu on device; host
    transposes the [1024,150] result.
  - Every dma_start stays <=256 packets (HWDGE ring depth) and triggers are
    spread across the sync/gpsimd/scalar/vector queues.
"""

import numpy as np

import concourse.bass as bass
import concourse.tile as tile
from concourse import bacc, mybir
from concourse.bass_utils import run_bass_kernel_spmd

N, D, R = 150, 1024, 92
NCORES = 8
P = 128
DC = D // P            # 8 contraction (d) chunks
EC = D // P            # 8 output (e) chunks
NB = EC // 2           # 4 main psum banks, two e-chunks each
NPAD = 256             # token axis padded so f32r moving >= 256
DEP_SLOTS = 12         # ceil(92/8)
SELF_SLOTS = 19        # ceil(150/8)
NSELF = NCORES * SELF_SLOTS  # 152 gathered self rows
KAUG = 32              # 12 dep-count rows + 19 one-hot rows + 1 pad
F32 = mybir.dt.float32
F32R = mybir.dt.float32r

DEP_SPLIT = [12, 12, 12, 12, 11, 11, 11, 11]
DEP_STARTS = np.concatenate([[0], np.cumsum(DEP_SPLIT)])

_PROG = None


def _build_program():
    nc = bacc.Bacc("TRN2", target_bir_lowering=False, debug=False, num_devices=NCORES)

    wdep = nc.dram_tensor("wdep", [DEP_SLOTS, D, D], F32R, kind="ExternalInput")
    wpos = nc.dram_tensor("wpos", [SELF_SLOTS, D, D], F32R, kind="ExternalInput")
    xs = nc.dram_tensor("xs", [DEP_SLOTS, DC, P, N], F32R, kind="ExternalInput")
    xtl = nc.dram_tensor("xtl", [DC, P, SELF_SLOTS], F32R, kind="ExternalInput")
    baug = nc.dram_tensor("baug", [KAUG, D], F32, kind="ExternalInput")
    caug = nc.dram_tensor("caug", [KAUG, NPAD], F32, kind="ExternalInput")
    # identity used to PE-transpose the gathered self rows: ident[g, j, n] = 1
    # iff n == 128*g + j
    ident = nc.dram_tensor("ident", [2, P, NPAD], F32R, kind="ExternalInput")
    out_T = nc.dram_tensor("out_T", [D, N], F32, kind="ExternalOutput")

    groups = [list(range(NCORES))]

    with tile.TileContext(nc) as tc:
        with (
            tc.tile_pool(name="wpool", bufs=4) as wpool,
            tc.tile_pool(name="xspool", bufs=1) as xspool,
            tc.tile_pool(name="constp", bufs=1) as constp,
            tc.tile_pool(name="mainps", bufs=1, space=bass.MemorySpace.PSUM) as mainps,
            tc.tile_pool(name="selfps", bufs=4, space=bass.MemorySpace.PSUM) as selfps,
            tc.tile_pool(name="dram", bufs=1, space="DRAM") as dram,
            tc.tile_pool(name="fin", bufs=3) as fin,
        ):
            xtl_t = constp.tile([P, DC * SELF_SLOTS], F32R)
            nc.gpsimd.dma_start(
                out=xtl_t.rearrange("p (c j) -> p c j", c=DC),
                in_=xtl[:].rearrange("c p j -> p c j"),
            )
            baug_t = constp.tile([KAUG, D], F32)
            nc.gpsimd.dma_start(out=baug_t[:], in_=baug[:])
            caug_t = constp.tile([KAUG, NPAD], F32)
            nc.gpsimd.dma_start(out=caug_t[:], in_=caug[:])
            ident_t = constp.tile([P, 2 * NPAD], F32R)
            nc.gpsimd.dma_start(
                out=ident_t.rearrange("p (g n) -> p g n", g=2),
                in_=ident[:].rearrange("g p n -> p g n"),
            )

            accs = [
                mainps.tile([P, 2 * NPAD], F32, name=f"acc{b}", tag=f"acc{b}")
                for b in range(NB)
            ]
            # Bias matmuls first: the single start=True per main PSUM bank (the
            # second region's first-touch rides the bank's pending-zero state).
            for b in range(NB):
                for h in range(2):
                    nc.tensor.matmul(
                        accs[b][:, h * NPAD : h * NPAD + NPAD],
                        baug_t[:, (2 * b + h) * P : (2 * b + h + 1) * P],
                        caug_t[:],
                        start=(h == 0),
                        stop=False,
                    )

            def load_w(src, u, eng):
                wt = wpool.tile([P, DC * D], F32R, tag="w", name=f"w{u}")
                src3 = src.rearrange("(c p) e -> p c e", p=P)
                wt3 = wt.rearrange("p (c e) -> p c e", c=DC)
                for g in range(4):
                    eng.dma_start(
                        out=wt3[:, 2 * g : 2 * g + 2, :],
                        in_=src3[:, 2 * g : 2 * g + 2, :],
                    )
                return wt

            # ---- self phase: 19 tokens, M=1 row formulation ----
            ar_self_in = dram.tile([SELF_SLOTS, D], F32R)
            ar_self_out = dram.tile([NCORES, SELF_SLOTS, D], F32R, addr_space="Shared")
            for j in range(SELF_SLOTS):
                wt = load_w(wpos[j], f"s{j}", nc.sync if j % 2 == 0 else nc.gpsimd)
                for eh in range(2):
                    st = selfps.tile([1, 512], F32, tag="sp", name=f"sp{j}_{eh}")
                    for c in range(DC):
                        nc.tensor.matmul(
                            st[:],
                            xtl_t[:, c * SELF_SLOTS + j : c * SELF_SLOTS + j + 1],
                            wt[:, c * D + eh * 512 : c * D + (eh + 1) * 512],
                            start=(c == 0),
                            stop=(c == DC - 1),
                        )
                    # ACT copy with f32r output = the "rounding" producer the
                    # BIR verifier wants for downstream f32r matmuls
                    sx = fin.tile([1, 512], F32R, tag="sx", name=f"sx{j}_{eh}")
                    nc.scalar.copy(out=sx[:], in_=st[:])
                    nc.scalar.dma_start(
                        out=ar_self_in[j : j + 1, eh * 512 : (eh + 1) * 512],
                        in_=sx[:],
                    )

            nc.gpsimd.collective_compute(
                "AllGather", mybir.AluOpType.bypass,
                replica_groups=groups, ins=[ar_self_in.opt()], outs=[ar_self_out.opt()],
            )

            # ---- dep phase: 12 type slots ----
            # xs tiles are pre-zeroed once; DMA refills only the first 150 of
            # each 256-wide chunk region, the zero padding is never rewritten.
            NXS = 3
            xsts = [xspool.tile([P, DC * NPAD], F32R, tag=f"xs{i}", name=f"xs{i}") for i in range(NXS)]
            for t in xsts:
                nc.vector.memset(t[:].bitcast(F32), 0.0)
            for i in range(DEP_SLOTS):
                wt = load_w(wdep[i], f"d{i}", nc.sync if i % 2 == 0 else nc.gpsimd)
                xst = xsts[i % NXS]
                xst3 = xst.rearrange("p (c f) -> p c f", c=DC)
                xsrc = xs[i].rearrange("c p f -> p c f")
                for g in range(4):
                    nc.scalar.dma_start(
                        out=xst3[:, 2 * g : 2 * g + 2, 0:N],
                        in_=xsrc[:, 2 * g : 2 * g + 2, :],
                    )
                last = i == DEP_SLOTS - 1
                for c in range(DC):
                    for ec in range(EC):
                        b, h = divmod(ec, 2)
                        nc.tensor.matmul(
                            accs[b][:, h * NPAD : h * NPAD + NPAD],
                            wt[:, c * D + ec * P : c * D + (ec + 1) * P],
                            xst[:, c * NPAD : (c + 1) * NPAD],
                            start=False,
                            stop=last and c == DC - 1 and h == 1,
                        )

            # ---- evacuate + AllReduce ----
            ar_main_in = dram.tile([D, N], F32)
            ar_main_out = dram.tile([D, N], F32, addr_space="Shared")
            for b in range(NB):
                ev = fin.tile([P, 2 * NPAD], F32, tag="ev", name=f"ev{b}")
                nc.vector.tensor_copy(ev[:], accs[b][:])
                for h in range(2):
                    nc.sync.dma_start(
                        out=ar_main_in[(2 * b + h) * P : (2 * b + h + 1) * P, :],
                        in_=ev[:, h * NPAD : h * NPAD + N],
                    )
            nc.gpsimd.collective_compute(
                "AllReduce", mybir.AluOpType.add,
                replica_groups=groups, ins=[ar_main_in.opt()], outs=[ar_main_out.opt()],
            )

            # ---- PE-transpose the gathered self rows into the freed banks ----
            # self_all is [(core k, j) = token 19k+j, e]; we need [e, token].
            # out_T_chunk[e, n] = sum_j self[jg*128+j, e] * ident[jg][j, n]
            sj0 = fin.tile([P, D], F32R, tag="sj0")
            sj1 = fin.tile([NSELF - P, D], F32R, tag="sj1")
            sflat = ar_self_out[:].rearrange("k j e -> (k j) e")
            nc.gpsimd.dma_start(out=sj0[:], in_=sflat[0:P, :])
            nc.gpsimd.dma_start(out=sj1[:], in_=sflat[P:NSELF, :])
            for ec in range(EC):
                b, h = divmod(ec, 2)
                nc.tensor.matmul(
                    accs[b][:, h * NPAD : h * NPAD + NPAD],
                    sj0[:, ec * P : (ec + 1) * P],
                    ident_t[:, 0:NPAD],
                    start=(h == 0),
                    stop=False,
                )
                nc.tensor.matmul(
                    accs[b][:, h * NPAD : h * NPAD + NPAD],
                    sj1[:, ec * P : (ec + 1) * P],
                    ident_t[0 : NSELF - P, NPAD : 2 * NPAD],
                    start=False,
                    stop=(h == 1),
                )

            # ---- final combine: out_T[e, n] = relu(main + self_T) ----
            for ec in range(EC):
                b, h = divmod(ec, 2)
                mc = fin.tile([P, N], F32, tag="mc", name=f"mc{ec}")
                nc.gpsimd.dma_start(out=mc[:], in_=ar_main_out[ec * P : (ec + 1) * P, :])
                oc = fin.tile([P, N], F32, tag="oc", name=f"oc{ec}")
                nc.vector.scalar_tensor_tensor(
                    oc[:], mc[:], 0.0, accs[b][:, h * NPAD : h * NPAD + N],
                    mybir.AluOpType.add, mybir.AluOpType.add,
                )
                nc.vector.tensor_scalar_max(oc[:], oc[:], 0.0)
                nc.sync.dma_start(out=out_T[ec * P : (ec + 1) * P, :], in_=oc[:])

    nc.compile()
    return nc


def _get_program():
    global _PROG
    if _PROG is None:
        _PROG = _build_program()
    return _PROG


def _prepare_in_maps(x, W_pos, b_pos, W_dep, b_dep, edge_token, edge_type):
    x = np.ascontiguousarray(np.asarray(x, dtype=np.float32))
    W_pos = np.asarray(W_pos, dtype=np.float32)
    b_pos = np.asarray(b_pos, dtype=np.float32)
    W_dep = np.asarray(W_dep, dtype=np.float32)
    b_dep = np.asarray(b_dep, dtype=np.float32)
    edge_token = np.asarray(edge_token)
    edge_type = np.asarray(edge_type)

    counts = np.zeros((N, R), np.float32)
    np.add.at(counts, (edge_token, edge_type), 1.0)
    xT = np.ascontiguousarray(x.T)  # [D, N]

    ident_np = np.zeros((2, P, NPAD), np.float32)
    for g in range(2):
        for j in range(P):
            n = g * P + j
            if n < NPAD:
                ident_np[g, j, n] = 1.0

    in_maps = []
    for k in range(NCORES):
        r0, r1 = int(DEP_STARTS[k]), int(DEP_STARTS[k + 1])
        nr = r1 - r0
        t0 = SELF_SLOTS * k
        t1 = min(t0 + SELF_SLOTS, N)
        nt = t1 - t0

        wdep_k = np.zeros((DEP_SLOTS, D, D), np.float32)
        wdep_k[:nr] = W_dep[r0:r1]
        wpos_k = np.zeros((SELF_SLOTS, D, D), np.float32)
        wpos_k[:nt] = W_pos[t0:t1]

        xs_k = np.zeros((DEP_SLOTS, DC, P, N), np.float32)
        for i in range(nr):
            xs_k[i] = (xT * counts[:, r0 + i][None, :]).reshape(DC, P, N)

        xtl_k = np.zeros((DC, P, SELF_SLOTS), np.float32)
        xtl_k[:, :, :nt] = xT[:, t0:t1].reshape(DC, P, nt)

        baug_k = np.zeros((KAUG, D), np.float32)
        baug_k[:nr] = b_dep[r0:r1]
        baug_k[DEP_SLOTS : DEP_SLOTS + nt] = b_pos[t0:t1]

        caug_k = np.zeros((KAUG, NPAD), np.float32)
        caug_k[:nr, 0:N] = counts[:, r0:r1].T
        for j in range(nt):
            caug_k[DEP_SLOTS + j, t0 + j] = 1.0

        in_maps.append(
            dict(wdep=wdep_k, wpos=wpos_k, xs=xs_k, xtl=xtl_k,
                 baug=baug_k, caug=caug_k, ident=ident_np)
        )
    return in_maps


def _run(in_maps, trace=False):
    nc = _get_program()
    return run_bass_kernel_spmd(nc, in_maps, list(range(NCORES)), trace=trace)


def kernel(x, W_pos, b_pos, W_dep, b_dep, edge_token, edge_type):
    in_maps = _prepare_in_maps(x, W_pos, b_pos, W_dep, b_dep, edge_token, edge_type)
    res = _run(in_maps, trace=False)
    return np.ascontiguousarray(res.results[0]["out_T"].T)


def kernel_traced(x, W_pos, b_pos, W_dep, b_dep, edge_token, edge_type):
    """Like kernel() but with NTFF profiling; returns (output, BassKernelResults)."""
    in_maps = _prepare_in_maps(x, W_pos, b_pos, W_dep, b_dep, edge_token, edge_type)
    res = _run(in_maps, trace=True)
    return np.ascontiguousarray(res.results[0]["out_T"].T), res


def install_ntff_shim():
    """The agent image's antenv lacks axon_hooks; recreate it from the boot
    module's ctypes NTFF driver so run_bass_kernel_spmd(trace=True) can
    capture a neuron-profile. Test-only; kernel() never needs this."""
    import sys
    import types

    try:
        from antenv.axon_hooks import get_axon_ntff_profile_hook  # noqa: F401
        return
    except ImportError:
        pass
    from trn_agent_boot.trn_boot import _ntff_profile_via_ctypes

    hook = _ntff_profile_via_ctypes("/opt/axon/libaxon_pjrt.so")
    mod = types.ModuleType("antenv.axon_hooks")
    mod._hook = hook
    mod.get_axon_ntff_profile_hook = lambda: mod._hook
    mod.set_axon_ntff_profile_hook = lambda h: setattr(mod, "_hook", h)
    sys.modules["antenv.axon_hooks"] = mod


# revision 17
# speedup vs baseline: 1.6382x; 1.0571x over previous
"""Trainium2 Bass kernel for the GCNN layer (nn_GCNNLayer_71536975282326).

out = relu( einsum('nd,nde->ne', x, W_pos) + b_pos
            + einsum('nre,nr->ne', einsum('nd,rde->nre', x, W_dep), counts)
            + counts @ b_dep )
with counts[n,r] = #edges (token n, type r).

Strategy (8 NeuronCores, SPMD, one program):
  - Shard the R=92 W_dep stack across cores (12 slots/core, zero-padded) and
    the N=150 W_pos stack across cores (19 slots/core, zero-padded).
  - All heavy matmuls run in float32r (fp32 storage, ~4x PE rate, measured
    ~1.4e-4 scale-relative error end to end).
  - Dep accumulation is transposed, out_T[e, n], in 4 PSUM banks (two 256-wide
    token regions per bank — f32r needs a >=256-wide moving operand for full
    rate). Moving operand is the host-prescaled (counts[:,r]*x)^T.
  - Self term: per token, 16 M=1 matmuls (W_pos[n_j] chunks moving 512-wide)
    into partition 0 of a small PSUM tile; the row is bounced through SBUF
    (engines cannot address partitions at j>0) into the AllGather input.
    Self units run FIRST so the AllGather overlaps the dep DMA stream.
  - The gathered [152, 1024] self matrix ([token, e]) is transposed back to
    [e, token] ON THE PE via identity matmuls into the freed PSUM banks —
    a strided-DMA transpose would degrade to 4-byte packets and flood the
    DMA engines (measured: 158K single-element packets, +300us).
  - Bias: one K=32 f32 matmul per e-chunk: lhsT rows = [b_dep slice ; b_pos
    rows], rhs = [counts slice^T ; one-hot placing token n_j at column n_j].
  - AllReduce the [1024,150] main partial; add + relu on device; host
    transposes the [1024,150] result.
  - Every dma_start stays <=256 packets (HWDGE ring depth) and triggers are
    spread across the sync/gpsimd/scalar/vector queues.
"""

import numpy as np

import concourse.bass as bass
import concourse.tile as tile
from concourse import bacc, mybir
from concourse.bass_utils import run_bass_kernel_spmd

N, D, R = 150, 1024, 92
NCORES = 8
P = 128
DC = D // P            # 8 contraction (d) chunks
EC = D // P            # 8 output (e) chunks
NB = EC // 2           # 4 main psum banks, two e-chunks each
NPAD = 256             # token axis padded so f32r moving >= 256
DEP_SLOTS = 12         # ceil(92/8)
SELF_SLOTS = 19        # ceil(150/8)
NSELF = NCORES * SELF_SLOTS  # 152 gathered self rows
KAUG = 32              # 12 dep-count rows + 19 one-hot rows + 1 pad
F32 = mybir.dt.float32
F32R = mybir.dt.float32r

DEP_SPLIT = [12, 12, 12, 12, 11, 11, 11, 11]
DEP_STARTS = np.concatenate([[0], np.cumsum(DEP_SPLIT)])

_PROG = None


def _build_program():
    nc = bacc.Bacc("TRN2", target_bir_lowering=False, debug=False, num_devices=NCORES)

    wdep = nc.dram_tensor("wdep", [DEP_SLOTS, D, D], F32R, kind="ExternalInput")
    wpos = nc.dram_tensor("wpos", [SELF_SLOTS, D, D], F32R, kind="ExternalInput")
    xs = nc.dram_tensor("xs", [DEP_SLOTS, DC, P, N], F32R, kind="ExternalInput")
    xtl = nc.dram_tensor("xtl", [DC, P, SELF_SLOTS], F32R, kind="ExternalInput")
    baug = nc.dram_tensor("baug", [KAUG, D], F32, kind="ExternalInput")
    caug = nc.dram_tensor("caug", [KAUG, NPAD], F32, kind="ExternalInput")
    # identity used to PE-transpose the gathered self rows: ident[g, j, n] = 1
    # iff n == 128*g + j
    ident = nc.dram_tensor("ident", [2, P, NPAD], F32R, kind="ExternalInput")
    # per-core output: this core's 128-row e-chunk of out_T (host assembles)
    out_T = nc.dram_tensor("out_T", [P, N], F32, kind="ExternalOutput")

    groups = [list(range(NCORES))]

    with tile.TileContext(nc) as tc:
        with (
            tc.tile_pool(name="wpool", bufs=3) as wpool,
            tc.tile_pool(name="xspool", bufs=1) as xspool,
            tc.tile_pool(name="constp", bufs=1) as constp,
            tc.tile_pool(name="mainps", bufs=1, space=bass.MemorySpace.PSUM) as mainps,
            tc.tile_pool(name="selfps", bufs=4, space=bass.MemorySpace.PSUM) as selfps,
            tc.tile_pool(name="dram", bufs=1, space="DRAM") as dram,
            tc.tile_pool(name="fin", bufs=3) as fin,
        ):
            xtl_t = constp.tile([P, DC * SELF_SLOTS], F32R)
            nc.gpsimd.dma_start(
                out=xtl_t.rearrange("p (c j) -> p c j", c=DC),
                in_=xtl[:].rearrange("c p j -> p c j"),
            )
            baug_t = constp.tile([KAUG, D], F32)
            nc.gpsimd.dma_start(out=baug_t[:], in_=baug[:])
            caug_t = constp.tile([KAUG, NPAD], F32)
            nc.gpsimd.dma_start(out=caug_t[:], in_=caug[:])
            ident_t = constp.tile([P, 2 * NPAD], F32R)
            nc.gpsimd.dma_start(
                out=ident_t.rearrange("p (g n) -> p g n", g=2),
                in_=ident[:].rearrange("g p n -> p g n"),
            )

            accs = [
                mainps.tile([P, 2 * NPAD], F32, name=f"acc{b}", tag=f"acc{b}")
                for b in range(NB)
            ]
            # Bias matmuls first: the single start=True per main PSUM bank (the
            # second region's first-touch rides the bank's pending-zero state).
            for b in range(NB):
                for h in range(2):
                    nc.tensor.matmul(
                        accs[b][:, h * NPAD : h * NPAD + NPAD],
                        baug_t[:, (2 * b + h) * P : (2 * b + h + 1) * P],
                        caug_t[:],
                        start=(h == 0),
                        stop=False,
                    )

            def load_w(src, u, eng):
                wt = wpool.tile([P, DC * D], F32R, tag="w", name=f"w{u}")
                src3 = src.rearrange("(c p) e -> p c e", p=P)
                wt3 = wt.rearrange("p (c e) -> p c e", c=DC)
                for g in range(4):
                    eng.dma_start(
                        out=wt3[:, 2 * g : 2 * g + 2, :],
                        in_=src3[:, 2 * g : 2 * g + 2, :],
                    )
                return wt

            # ---- self phase: 19 tokens, M=1 row formulation ----
            ar_self_in = dram.tile([SELF_SLOTS, D], F32R)
            ar_self_out = dram.tile([NCORES, SELF_SLOTS, D], F32R, addr_space="Shared")
            for j in range(SELF_SLOTS):
                wt = load_w(wpos[j], f"s{j}", nc.sync if j % 2 == 0 else nc.gpsimd)
                for eh in range(2):
                    st = selfps.tile([1, 512], F32, tag="sp", name=f"sp{j}_{eh}")
                    for c in range(DC):
                        nc.tensor.matmul(
                            st[:],
                            xtl_t[:, c * SELF_SLOTS + j : c * SELF_SLOTS + j + 1],
                            wt[:, c * D + eh * 512 : c * D + (eh + 1) * 512],
                            start=(c == 0),
                            stop=(c == DC - 1),
                        )
                    # ACT copy with f32r output = the "rounding" producer the
                    # BIR verifier wants for downstream f32r matmuls
                    sx = fin.tile([1, 512], F32R, tag="sx", name=f"sx{j}_{eh}")
                    nc.scalar.copy(out=sx[:], in_=st[:])
                    nc.scalar.dma_start(
                        out=ar_self_in[j : j + 1, eh * 512 : (eh + 1) * 512],
                        in_=sx[:],
                    )

            nc.gpsimd.collective_compute(
                "AllGather", mybir.AluOpType.bypass,
                replica_groups=groups, ins=[ar_self_in.opt()], outs=[ar_self_out.opt()],
            )

            # ---- dep phase: 12 type slots ----
            # xs tiles are pre-zeroed once; DMA refills only the first 150 of
            # each 256-wide chunk region, the zero padding is never rewritten.
            NXS = 3
            xsts = [xspool.tile([P, DC * NPAD], F32R, tag=f"xs{i}", name=f"xs{i}") for i in range(NXS)]
            for t in xsts:
                nc.vector.memset(t[:].bitcast(F32), 0.0)
            for i in range(DEP_SLOTS):
                wt = load_w(wdep[i], f"d{i}", nc.sync if i % 2 == 0 else nc.gpsimd)
                xst = xsts[i % NXS]
                xst3 = xst.rearrange("p (c f) -> p c f", c=DC)
                xsrc = xs[i].rearrange("c p f -> p c f")
                for g in range(4):
                    nc.scalar.dma_start(
                        out=xst3[:, 2 * g : 2 * g + 2, 0:N],
                        in_=xsrc[:, 2 * g : 2 * g + 2, :],
                    )
                last = i == DEP_SLOTS - 1
                for c in range(DC):
                    for ec in range(EC):
                        b, h = divmod(ec, 2)
                        nc.tensor.matmul(
                            accs[b][:, h * NPAD : h * NPAD + NPAD],
                            wt[:, c * D + ec * P : c * D + (ec + 1) * P],
                            xst[:, c * NPAD : (c + 1) * NPAD],
                            start=False,
                            stop=last and c == DC - 1 and h == 1,
                        )

            # ---- evacuate + ReduceScatter (core k receives e-chunk k) ----
            ar_main_in = dram.tile([D, N], F32)
            rs_out = dram.tile([P, N], F32)
            for b in range(NB):
                ev = fin.tile([P, 2 * NPAD], F32, tag="ev", name=f"ev{b}")
                nc.vector.tensor_copy(ev[:], accs[b][:])
                for h in range(2):
                    nc.sync.dma_start(
                        out=ar_main_in[(2 * b + h) * P : (2 * b + h + 1) * P, :],
                        in_=ev[:, h * NPAD : h * NPAD + N],
                    )
            nc.gpsimd.collective_compute(
                "ReduceScatter", mybir.AluOpType.add,
                replica_groups=groups, ins=[ar_main_in.opt()], outs=[rs_out.opt()],
            )

            # ---- PE-transpose the gathered self rows into the freed banks ----
            # self_all is [(core k, j) = token 19k+j, e]; we need [e, token].
            # out_T_chunk[e, n] = sum_j self[jg*128+j, e] * ident[jg][j, n]
            sj0 = constp.tile([P, D], F32R, tag="sj0")
            sj1 = constp.tile([NSELF - P, D], F32R, tag="sj1")
            sflat = ar_self_out[:].rearrange("k j e -> (k j) e")
            nc.gpsimd.dma_start(out=sj0[:], in_=sflat[0:P, :])
            nc.gpsimd.dma_start(out=sj1[:], in_=sflat[P:NSELF, :])
            for ec in range(EC):
                b, h = divmod(ec, 2)
                nc.tensor.matmul(
                    accs[b][:, h * NPAD : h * NPAD + NPAD],
                    sj0[:, ec * P : (ec + 1) * P],
                    ident_t[:, 0:NPAD],
                    start=(h == 0),
                    stop=False,
                )
                nc.tensor.matmul(
                    accs[b][:, h * NPAD : h * NPAD + NPAD],
                    sj1[:, ec * P : (ec + 1) * P],
                    ident_t[0 : NSELF - P, NPAD : 2 * NPAD],
                    start=False,
                    stop=(h == 1),
                )

            # ---- final combine (own e-chunk only): out_chunk = relu(rs + self_T) ----
            # selfT for ALL chunks sits in PSUM (the transpose is cheap and
            # keeps the program SPMD-uniform); this core's chunk is selected
            # with a partition_id-driven dynamic slice.
            selfT_sb = constp.tile([P, NB * 2 * NPAD], F32, tag="sT")
            for b in range(NB):
                nc.vector.tensor_copy(
                    selfT_sb[:, b * 2 * NPAD : (b + 1) * 2 * NPAD], accs[b][:]
                )
            pid = nc.vector.partition_id()
            col0 = pid * NPAD
            mc = fin.tile([P, N], F32, tag="mc")
            nc.gpsimd.dma_start(out=mc[:], in_=rs_out[:])
            oc = fin.tile([P, N], F32, tag="oc")
            nc.vector.scalar_tensor_tensor(
                oc[:], mc[:], 0.0, selfT_sb[:, bass.ds(col0, N)],
                mybir.AluOpType.add, mybir.AluOpType.add,
            )
            nc.vector.tensor_scalar_max(oc[:], oc[:], 0.0)
            nc.sync.dma_start(out=out_T[:], in_=oc[:])

    nc.compile()
    return nc


def _get_program():
    global _PROG
    if _PROG is None:
        _PROG = _build_program()
    return _PROG


def _prepare_in_maps(x, W_pos, b_pos, W_dep, b_dep, edge_token, edge_type):
    x = np.ascontiguousarray(np.asarray(x, dtype=np.float32))
    W_pos = np.asarray(W_pos, dtype=np.float32)
    b_pos = np.asarray(b_pos, dtype=np.float32)
    W_dep = np.asarray(W_dep, dtype=np.float32)
    b_dep = np.asarray(b_dep, dtype=np.float32)
    edge_token = np.asarray(edge_token)
    edge_type = np.asarray(edge_type)

    counts = np.zeros((N, R), np.float32)
    np.add.at(counts, (edge_token, edge_type), 1.0)
    xT = np.ascontiguousarray(x.T)  # [D, N]

    ident_np = np.zeros((2, P, NPAD), np.float32)
    for g in range(2):
        for j in range(P):
            n = g * P + j
            if n < NPAD:
                ident_np[g, j, n] = 1.0

    in_maps = []
    for k in range(NCORES):
        r0, r1 = int(DEP_STARTS[k]), int(DEP_STARTS[k + 1])
        nr = r1 - r0
        t0 = SELF_SLOTS * k
        t1 = min(t0 + SELF_SLOTS, N)
        nt = t1 - t0

        wdep_k = np.zeros((DEP_SLOTS, D, D), np.float32)
        wdep_k[:nr] = W_dep[r0:r1]
        wpos_k = np.zeros((SELF_SLOTS, D, D), np.float32)
        wpos_k[:nt] = W_pos[t0:t1]

        xs_k = np.zeros((DEP_SLOTS, DC, P, N), np.float32)
        for i in range(nr):
            xs_k[i] = (xT * counts[:, r0 + i][None, :]).reshape(DC, P, N)

        xtl_k = np.zeros((DC, P, SELF_SLOTS), np.float32)
        xtl_k[:, :, :nt] = xT[:, t0:t1].reshape(DC, P, nt)

        baug_k = np.zeros((KAUG, D), np.float32)
        baug_k[:nr] = b_dep[r0:r1]
        baug_k[DEP_SLOTS : DEP_SLOTS + nt] = b_pos[t0:t1]

        caug_k = np.zeros((KAUG, NPAD), np.float32)
        caug_k[:nr, 0:N] = counts[:, r0:r1].T
        for j in range(nt):
            caug_k[DEP_SLOTS + j, t0 + j] = 1.0

        in_maps.append(
            dict(wdep=wdep_k, wpos=wpos_k, xs=xs_k, xtl=xtl_k,
                 baug=baug_k, caug=caug_k, ident=ident_np)
        )
    return in_maps


def _run(in_maps, trace=False):
    nc = _get_program()
    return run_bass_kernel_spmd(nc, in_maps, list(range(NCORES)), trace=trace)


def _assemble(res):
    out_T = np.concatenate([res.results[k]["out_T"] for k in range(NCORES)], axis=0)
    return np.ascontiguousarray(out_T.T)


def kernel(x, W_pos, b_pos, W_dep, b_dep, edge_token, edge_type):
    in_maps = _prepare_in_maps(x, W_pos, b_pos, W_dep, b_dep, edge_token, edge_type)
    res = _run(in_maps, trace=False)
    return _assemble(res)


def kernel_traced(x, W_pos, b_pos, W_dep, b_dep, edge_token, edge_type):
    """Like kernel() but with NTFF profiling; returns (output, BassKernelResults)."""
    in_maps = _prepare_in_maps(x, W_pos, b_pos, W_dep, b_dep, edge_token, edge_type)
    res = _run(in_maps, trace=True)
    return _assemble(res), res


def install_ntff_shim():
    """The agent image's antenv lacks axon_hooks; recreate it from the boot
    module's ctypes NTFF driver so run_bass_kernel_spmd(trace=True) can
    capture a neuron-profile. Test-only; kernel() never needs this."""
    import sys
    import types

    try:
        from antenv.axon_hooks import get_axon_ntff_profile_hook  # noqa: F401
        return
    except ImportError:
        pass
    from trn_agent_boot.trn_boot import _ntff_profile_via_ctypes

    hook = _ntff_profile_via_ctypes("/opt/axon/libaxon_pjrt.so")
    mod = types.ModuleType("antenv.axon_hooks")
    mod._hook = hook
    mod.get_axon_ntff_profile_hook = lambda: mod._hook
    mod.set_axon_ntff_profile_hook = lambda h: setattr(mod, "_hook", h)
    sys.modules["antenv.axon_hooks"] = mod
